# revision 42
# baseline (speedup 1.0000x reference)
"""Trainium2 Bass kernel for nn_CCG_46273977647541.

Reference pipeline per batch (B=8 -> one NeuronCore each, no cross-core
communication): LayerNorm -> NxN cosine similarity -> density row-sum ->
argmax row as cluster center -> 256->64 projection + relu.

The NxN similarity is never materialized.  With ln_w==1, ln_b==0 (the
spec's deterministic fills) the density factorizes exactly through the
CENTERED rows xc_n = x_n - mu_n:

  u_n       = xc_n / |xc_n|,   |xc_n| = sqrt(C*var_n)
  density_n = u_n . sum_m u_m = q_n * (xc_n . S),  S = sum_m q_m xc_m
  q_n       = rsqrt(C*var_n)

(The 1e-8 in the reference cosine denominator is a ~4e-11 relative
perturbation and is dropped; |x-hat| cancels out of u_n entirely.)

Centering during the f32->bf16 cast (ScalarE activation with per-partition
bias = -mu) removes every mu-correction from the baseline: no extra
matmul columns, no sum(S) terms, no tail subtractions.

Schedule (per core, engine-parallel):
  DMA   x streamed in 9 chunks on the single SP HWDGE ring (one FIFO
        drains at full SDMA rate; multi-ring splits round-robin and
        delay every chunk).  Small leading/trailing chunks shorten the
        pipeline head and the post-load critical chain.
  DVE   per-tile bn_stats + per-group half-merge -> q; then 22 of the
        32 dot tiles (STT+accum, the 1x-mode floor at ~346ns/tile).
  ACT   centered casts (Identity, bias=-mu), sqrt's; then 10 dot tiles
        via the square expansion  xc.S = (sum(xc+S)^2 - C var - sum S^2)/2
        (Square+accum over PE-built z=xc+S PSUM pairs) -- exact in the
        same bf16 inputs, so the two dot paths agree to ~5e-4.
  PE    warmup chain (HAM clock-gate release: cold 1.2GHz -> warm 2.4GHz
        before the S chain), S matmuls, z pairs, transposes, projections.

Numerics: bf16 dots/matmuls with f32 accumulation, f32 stats and q.
Measured density error vs exact f32 on the spec inputs ~0.05 against a
minimum top-2 gap of 0.26; end-to-end relative error ~2e-3 (gate 2e-2).

Infrastructure notes: this walrus build accepts only ONE semaphore wait
per engine instruction and rejects some custom ISA ops; _split_multi_waits
post-processes the BIR JSON to hoist extra waits onto EventSemaphore
carriers and neutralize non-fatal SeqAsserts.
"""

import sys

sys.path.insert(0, "/opt/trn_rl_repo")

from contextlib import ExitStack

import numpy as np

import concourse.bass as bass
import concourse.tile as tile
from concourse import mybir
from concourse.bass_utils import run_bass_kernel_spmd
from concourse.tile import add_dep_helper

F32 = mybir.dt.float32
BF16 = mybir.dt.bfloat16
AX = mybir.AxisListType
OP = mybir.AluOpType
ACT = mybir.ActivationFunctionType


def _split_multi_waits(bir_json: bytes) -> bytes:
    """This walrus build accepts at most one semaphore wait per engine
    instruction.  Tile can emit several; hoist all but the last onto
    dedicated EventSemaphore carriers placed immediately before the
    instruction (same engine stream, so semantics are preserved --
    the block order is a topological order of the dep graph)."""
    import json as _json

    bir = _json.loads(bir_json)
    n = 0
    for fn in bir["functions"]:
        for bb in fn["blocks"]:
            new = []
            for inst in bb["instructions"]:
                if inst.get("op_name") == "SeqAssert":
                    inst = {
                        "debug": inst.get("debug", 0),
                        "engine": inst["engine"],
                        "ins": [],
                        "outs": [],
                        "name": inst["name"],
                        "opcode": "EventSemaphore",
                        "sync_info": inst.get("sync_info")
                        or {"on_update": [], "on_wait": []},
                    }
                si = inst.get("sync_info")
                waits = (si or {}).get("on_wait") or []
                if len(waits) > 1:
                    for w in waits[:-1]:
                        n += 1
                        new.append(
                            {
                                "debug": inst.get("debug", 0),
                                "engine": inst["engine"],
                                "ins": [],
                                "outs": [],
                                "name": f"antsplitw-{n}",
                                "opcode": "EventSemaphore",
                                "sync_info": {"on_update": [], "on_wait": [w]},
                            }
                        )
                    si["on_wait"] = [waits[-1]]
                new.append(inst)
            bb["instructions"] = new
    return _json.dumps(bir).encode()


def _install_wait_splitter():
    from concourse import bass_utils as _bu
    from concourse import bass2jax as _b2j

    if getattr(_bu, "_ant_wait_splitter", False):
        return
    _orig = _bu.compile_bir_kernel

    def _patched(bir_json, tmpdir, neff_name="file.neff"):
        return _orig(_split_multi_waits(bir_json), tmpdir, neff_name)

    _bu.compile_bir_kernel = _patched
    _bu._ant_wait_splitter = True
    if getattr(_b2j, "compile_bir_kernel", None) is _orig:
        _b2j.compile_bir_kernel = _patched


_install_wait_splitter()

B, N, C, CR = 8, 4096, 256, 64
P = 128
NT = N // P  # 32 row tiles per core
LN_EPS = 1e-5

_CACHE: dict = {}


def _build_nc() -> bass.Bass:
    nc = bass.Bass(enable_asserts=False)
    x_d = nc.declare_dram_parameter("x", [N, C], F32, isOutput=False)
    pw_d = nc.declare_dram_parameter("proj_w", [CR, C], F32, isOutput=False)
    pb_d = nc.declare_dram_parameter("proj_b", [CR], F32, isOutput=False)
    out_d = nc.declare_dram_parameter("out", [CR], F32, isOutput=True)

    with ExitStack() as ctx:
        tc = ctx.enter_context(tile.TileContext(nc))
        small = ctx.enter_context(tc.tile_pool(name="small", bufs=1))
        scrp = ctx.enter_context(tc.tile_pool(name="scr", bufs=6))
        psum = ctx.enter_context(tc.tile_pool(name="ps", bufs=1, space="PSUM"))
        zpool = ctx.enter_context(tc.tile_pool(name="z", bufs=2, space="PSUM"))

        # Row n of this core's batch lives at (partition n//NT, tile n%NT):
        # partition-major so each DMA descriptor reads contiguous DRAM.
        xbig = small.tile([P, NT, C], F32)
        xb16 = small.tile([P, NT, C], BF16)
        ST6 = small.tile([P, NT, 6], F32)  # per-tile even/odd half stats
        DD = small.tile([P, NT], F32)
        D2 = small.tile([P, NT], F32)
        MSM2 = small.tile([P, NT, 2], F32)
        VA = small.tile([P, NT], F32)
        NMU = small.tile([P, NT], F32)  # -mu (cast bias)
        QS = small.tile([P, NT], F32)
        RS = small.tile([P, NT], F32)
        QQ = small.tile([P, NT], F32)
        RR = small.tile([P, NT], F32)
        QQb = small.tile([P, NT], BF16)
        XS = small.tile([P, NT], F32)
        DEN = small.tile([P, NT], F32)
        MASK = small.tile([P, NT], F32)
        MASKP = small.tile([P, NT], F32)
        pmrow = small.tile([1, P], F32)
        pscr = small.tile([1, P], F32)
        jstar = small.tile([1, 1], F32)
        W1 = small.tile([P, NT], F32)
        w1sel = small.tile([P, 1], F32)
        w1sel16 = small.tile([P, 1], BF16)
        S_row16 = small.tile([1, 2 * C], BF16)
        Sb16 = small.tile([P, C], BF16)
        Id16 = small.tile([P, P], BF16)
        ssq_scr = small.tile([1, C], BF16)
        ssS1 = small.tile([1, 1], F32)
        TQS = small.tile([P, NT], F32)
        XSQ = small.tile([P, NT], F32)
        CORR = small.tile([P, NT], F32)
        dmax = small.tile([P, 1], F32)
        gm1 = small.tile([1, 1], F32)
        pw_sb = small.tile([CR, C], F32)
        pb_row = small.tile([1, CR], F32)
        o_sb = small.tile([CR, 1], F32)
        o_row = small.tile([1, CR], F32)
        cen16 = small.tile([1, C], BF16)
        warm = small.tile([1, 1], F32)
        eps_sb = small.tile([P, 1], F32)
        ones_sb = small.tile([1, P], F32)
        halves_sb = small.tile([1, P], F32)
        ones16 = small.tile([1, P], BF16)
        wdum16 = small.tile([1, P], BF16)
        id_sb = small.tile([P, P], F32)
        ii32 = small.tile([P, P], mybir.dt.int32)
        ji32 = small.tile([P, NT], mybir.dt.int32)
        IOTAJ = small.tile([P, NT], F32)
        JIDX = small.tile([P, 1], F32)
        j32 = small.tile([1, 1], mybir.dt.int32)
        scrj = small.tile([P, NT], F32)
        pi32 = small.tile([P, 1], mybir.dt.int32)
        iif = small.tile([P, P], F32)
        pif = small.tile([P, 1], F32)

        scr2 = small.tile([CR, C], F32)
        S_ps = psum.tile([1, C], F32)
        wup_ps = psum.tile([P, P], F32, tag="wup")
        dmy_ps = psum.tile([1, 1], F32, tag="dmy")
        sb_ps = psum.tile([P, C], F32, tag="sb")
        cen_ps = psum.tile([CR, C], F32, tag="cen")

        # ---- Constants (DVE/GpSimd, before data lands) ----
        xv = x_d[:, :].rearrange("(p j) c -> p j c", p=P)
        nc.vector.memset(warm, 1.0)
        nc.vector.memset(eps_sb, LN_EPS)
        nc.vector.memset(ones_sb, 1.0)
        nc.vector.memset(halves_sb, 0.5)
        nc.vector.memset(ones16, 1.0)
        nc.vector.memset(wdum16, 0.0)
        nc.gpsimd.iota(ii32, pattern=[[1, P]], base=0, channel_multiplier=0)
        nc.gpsimd.iota(ji32, pattern=[[1, NT]], base=0, channel_multiplier=0)
        nc.vector.tensor_copy(IOTAJ, ji32)
        nc.gpsimd.iota(pi32, pattern=[[0, 1]], base=0, channel_multiplier=1)
        nc.vector.tensor_copy(iif, ii32)
        nc.vector.tensor_copy(pif, pi32)
        nc.vector.tensor_scalar(
            out=id_sb, in0=iif, scalar1=pif, scalar2=None, op0=OP.is_equal
        )
        nc.vector.tensor_copy(Id16, id_sb)

        # ---- x DMA: all chunks on the single SP HWDGE ring (one FIFO
        # drains each chunk at full SDMA rate; interleaving rings makes
        # the SDMA engines round-robin and delays every chunk).
        CBND = [0, 2, 6, 10, 14, 18, 22, 26, 30, 32]
        for c in range(len(CBND) - 1):
            sl = slice(CBND[c], CBND[c + 1])
            nc.sync.dma_start(out=xbig[:, sl, :], in_=xv[:, sl, :])
        nc.gpsimd.dma_start(out=pw_sb, in_=pw_d[:, :])
        nc.gpsimd.dma_start(out=pb_row, in_=pb_d[None, :])

        # ACT table load (Sqrt) after the chunk-0 issue
        nc.scalar.activation(out=warm, in_=warm, func=ACT.Sqrt)

        # ---- PE warmup: release the HAM clock gate (~3.4us of sustained
        # activity -> 1.2GHz cold to 2.4GHz warm) before the S-chain.
        wu = nc.tensor.matmul(
            wup_ps[:, :], ones16[0:1, :], wdum16[0:1, :], start=True, stop=False
        )
        for _ in range(23):
            wu = nc.tensor.matmul(
                wup_ps[:, :], ones16[0:1, :], wdum16[0:1, :], start=False, stop=False
            )
        nc.tensor.matmul(
            wup_ps[:, :], ones16[0:1, :], wdum16[0:1, :], start=False, stop=True
        )

        # ---- Phase 1: per chunk (DVE in program order, so stats/merge/cast
        # interleave chunk by chunk): bn_stats -> merge -> q -> centered
        # cast -> S matmuls.  The last groups put casts on DVE so the
        # critical tail doesn't wait for the (busier) ACT cast queue.
        GRPS = [0, 2, 10, 18, 26, 30, 32]
        NG = len(GRPS) - 1
        for g in range(NG):
            g0, g1 = GRPS[g], GRPS[g + 1]
            sl = slice(g0, g1)
            # per-tile stats (this build rejects multi-slice bn_stats)
            for h in range(g0, g1):
                nc.vector.bn_stats(out=ST6[:, h, :], in_=xbig[:, h, :])
            me, mo = ST6[:, sl, 1], ST6[:, sl, 4]
            # MSM2[:, :, 0] = me+mo (2*mu), MSM2[:, :, 1] = M2e+M2o -- one
            # strided TT covers both halves' merge adds
            nc.vector.tensor_add(
                MSM2[:, sl, :], ST6[:, sl, 1:3], ST6[:, sl, 4:6]
            )
            # var = (M2e+M2o)/C + ((me-mo)/2)^2 ; -mu = -(me+mo)/2
            nc.vector.tensor_sub(DD[:, sl], me, mo)
            nc.vector.scalar_tensor_tensor(
                out=D2[:, sl], in0=DD[:, sl], scalar=0.25, in1=DD[:, sl],
                op0=OP.mult, op1=OP.mult,
            )
            nc.vector.scalar_tensor_tensor(
                out=VA[:, sl], in0=MSM2[:, sl, 1], scalar=1.0 / C, in1=D2[:, sl],
                op0=OP.mult, op1=OP.add,
            )
            nc.vector.tensor_scalar_mul(NMU[:, sl], MSM2[:, sl, 0], -0.5)
            if g == NG - 1:
                # final group's centered casts on DVE, hoisted before the
                # sqrt/recip so they overlap the (queued) ACT sqrt
                for j in range(g0, g1):
                    nc.vector.tensor_scalar(
                        out=xb16[:, j, :], in0=xbig[:, j, :],
                        scalar1=NMU[:, j : j + 1], scalar2=None, op0=OP.add,
                    )
            # ACT casts for the group two back are emitted BEFORE this
            # group's Sqrt: they are always ready (their NMU is two groups
            # old), so they never sit blocked behind a not-yet-ready Sqrt
            # in ACT's in-order queue -- and each Sqrt is only ever behind
            # already-runnable casts.
            emit_groups = [g - 2] if g >= 2 else []
            if g == NG - 1:
                emit_groups.extend([g - 1, g])
            # q = rsqrt(C*var)
            nc.scalar.activation(
                out=QS[:, sl], in_=VA[:, sl], func=ACT.Sqrt, scale=float(C)
            )
            nc.vector.reciprocal(out=QQ[:, sl], in_=QS[:, sl])
            nc.vector.tensor_copy(QQb[:, sl], QQ[:, sl])
            for ge in emit_groups:
                e0, e1 = GRPS[ge], GRPS[ge + 1]
                if ge < NG - 1:
                    for j in range(e0, e1):
                        nc.scalar.activation(
                            out=xb16[:, j, :], in_=xbig[:, j, :],
                            func=ACT.Identity, bias=NMU[:, j : j + 1],
                        )
                # PE pre-join on DVE so real matmuls carry only one wait
                dmy = nc.tensor.matmul(
                    dmy_ps[:, :], QQb[:, e0 : e0 + 1], QQb[:, e0 : e0 + 1],
                    start=True, stop=True,
                )
                for j in range(e0, e1):
                    mm1 = nc.tensor.matmul(
                        S_ps[:, :], QQb[:, j : j + 1], xb16[:, j, :],
                        start=(j == 0), stop=(j == NT - 1),
                    )
                    add_dep_helper(mm1.ins, dmy.ins, False, "pe-prejoin")

        # ---- S finalize + broadcast (DVE casts S twice into a [1, 2C]
        # row so one PE matmul feeds both the broadcast and the paired
        # z-tiles of the square-trick below) ----
        nc.scalar.copy(out=S_row16[0:1, 0:C], in_=S_ps[0:1, :])
        nc.vector.tensor_copy(S_row16[0:1, C : 2 * C], S_ps[0:1, :])
        nc.tensor.matmul(
            sb_ps[:, :], ones16[0:1, :], S_row16[0:1, 0:C], start=True, stop=True
        )
        nc.vector.tensor_copy(Sb16, sb_ps[:, :])

        # r = rsqrt(var+eps) for the mask weights (overlaps the dot pass:
        # ACT sqrt right away, DVE recip interleaved mid-dots)
        nc.scalar.activation(
            out=RS[:, :], in_=VA[:, :], func=ACT.Sqrt, bias=eps_sb[:, 0:1]
        )

        # ---- Phase 2: per-row dot xc_n . S ----
        # Tiles 0..NSQ-1 go through the square-expansion on the otherwise
        # idle PE+ACT engines:  xc.S = (sum(xc+S)^2 - C*var - sum S^2)/2,
        # with z = xc + S formed in PSUM by two matmuls per tile pair
        # (identity @ xc accumulated with ones x S) -- all f32-exact in
        # the bf16 inputs.  Tiles NSQ.. stay on the DVE STT+accum path.
        NSQ = 10
        # sum S^2 (scaled 0.5) on DVE -- overlaps the ACT S_row copy
        nc.vector.scalar_tensor_tensor(
            out=ssq_scr, in0=S_row16[0:1, 0:C], scalar=1.0,
            in1=S_row16[0:1, 0:C], op0=OP.mult, op1=OP.mult, accum_out=ssS1,
        )
        nc.vector.tensor_scalar_mul(ssS1, ssS1, 0.5)
        # PE stream: the z pairs first (they gate the ACT squares), then
        # the sum-S^2 broadcast for the correction term.
        ssb_ps = psum.tile([P, 1], F32, tag="mx")
        for k in range(0, NSQ, 2):
            zp = zpool.tile([P, 2 * C], F32, tag="z")
            nc.tensor.matmul(
                zp[:, :], Id16[:, :], xb16[:, k : k + 2, :], start=True, stop=False
            )
            nc.tensor.matmul(
                zp[:, :], ones16[0:1, :], S_row16[0:1, :], start=False, stop=True
            )
            for t in range(2):
                scr = scrp.tile([P, C], BF16, tag="sqr")
                nc.scalar.activation(
                    out=scr, in_=zp[:, t * C : (t + 1) * C], func=ACT.Square,
                    accum_out=XSQ[:, k + t : k + t + 1],
                )
        nc.tensor.matmul(
            ssb_ps[:, :], ones_sb[0:1, :], ssS1[0:1, 0:1], start=True, stop=True
        )
        for j in range(NSQ, NT):
            scr = scrp.tile([P, C], BF16, tag="scr")
            nc.vector.scalar_tensor_tensor(
                out=scr, in0=xb16[:, j, :], scalar=1.0, in1=Sb16,
                op0=OP.mult, op1=OP.mult, accum_out=XS[:, j : j + 1],
            )
            if j == NSQ + 4:
                # correction CORR = 0.5*(C*var) + 0.5*sum S^2  (C*var = QS^2)
                nc.vector.scalar_tensor_tensor(
                    out=TQS, in0=QS, scalar=1.0, in1=QS, op0=OP.mult, op1=OP.mult
                )
            if j == NSQ + 7:
                nc.vector.tensor_scalar(
                    out=CORR, in0=TQS, scalar1=0.5, scalar2=ssb_ps[:, 0:1],
                    op0=OP.mult, op1=OP.add,
                )
            if j == NSQ + 10:
                nc.vector.reciprocal(out=RR[:, :], in_=RS[:, :])
        # finalize the square-trick columns: XS = 0.5*ssq - CORR
        nc.vector.scalar_tensor_tensor(
            out=XS[:, 0:NSQ], in0=XSQ[:, 0:NSQ], scalar=0.5, in1=CORR[:, 0:NSQ],
            op0=OP.mult, op1=OP.subtract,
        )

        # density (up to the global positive scale q_n applied per row)
        nc.vector.tensor_mul(DEN, XS, QQ)

        # ---- Phase 3: global argmax.  Two parallel chains:
        #   A (j*): per-partition local mask/argmax-idx -> transpose both
        #     to partition-0 rows -> winner-row one-hot -> j* -> value_load
        #   B (weights): gmax broadcast -> global mask -> w1sel
        # A's slow value_load+register-hazard overlaps B's mask math.
        nc.vector.reduce_max(out=dmax, in_=DEN, axis=AX.X)
        nc.vector.tensor_scalar(
            out=MASKP, in0=DEN, scalar1=dmax[:, 0:1], scalar2=None, op0=OP.is_equal
        )
        nc.vector.scalar_tensor_tensor(
            out=scrj, in0=MASKP, scalar=1.0, in1=IOTAJ,
            op0=OP.mult, op1=OP.mult, accum_out=JIDX,
        )
        tr_ps = psum.tile([1, P], F32, tag="mx")
        nc.tensor.transpose(tr_ps[:, :], dmax[:, 0:1], id_sb[:, :])
        trb_ps = psum.tile([1, P], F32, tag="cen")
        nc.tensor.transpose(trb_ps[:, :], JIDX[:, 0:1], id_sb[:, :])
        nc.vector.reduce_max(out=gm1, in_=tr_ps[0:1, :], axis=AX.X)
        nc.vector.tensor_scalar(
            out=pmrow, in0=tr_ps[0:1, :], scalar1=gm1[0:1, 0:1], scalar2=None,
            op0=OP.is_equal,
        )
        nc.vector.scalar_tensor_tensor(
            out=pscr, in0=pmrow, scalar=1.0, in1=trb_ps[0:1, :],
            op0=OP.mult, op1=OP.mult, accum_out=jstar,
        )
        with nc.allow_low_precision(reason="exact small-int index value"):
            nc.vector.tensor_copy(j32, jstar)
        jv = nc.tensor.value_load(j32[0:1, 0:1])
        gmax_ps = psum.tile([P, 1], F32, tag="mx")
        nc.tensor.matmul(
            gmax_ps[:, :], ones_sb[0:1, :], gm1[0:1, 0:1], start=True, stop=True
        )
        nc.vector.tensor_scalar(
            out=MASK, in0=DEN, scalar1=gmax_ps[:, 0:1], scalar2=None, op0=OP.is_equal
        )
        nc.vector.tensor_mul(W1, MASK, RR)
        nc.vector.reduce_sum(out=w1sel, in_=W1, axis=AX.X)
        nc.vector.tensor_copy(w1sel16, w1sel)

        # ---- Phase 4: center = sum_p w1[p,j*] xc[p,j*,:] ----
        cc_ps = psum.tile([1, C], F32, tag="mx")
        nc.tensor.matmul(
            cc_ps[:, :],
            w1sel16[:, 0:1],
            xb16[:, bass.ds(jv, 1), :],
            start=True,
            stop=True,
        )
        nc.scalar.copy(out=cen16, in_=cc_ps[0:1, :])

        # ---- Phase 5: out = relu(proj_w @ center + proj_b) ----
        nc.tensor.matmul(
            cen_ps[:, :], ones16[0:1, 0:CR], cen16[0:1, :], start=True, stop=True
        )
        nc.vector.scalar_tensor_tensor(
            out=scr2, in0=pw_sb, scalar=1.0, in1=cen_ps[:, :],
            op0=OP.mult, op1=OP.mult, accum_out=o_sb,
        )
        # transpose [64,1] -> [1,64] so the output DMA is one contiguous
        # 256B descriptor instead of 64 partition-strided 4B writes
        o_ps = psum.tile([1, CR], F32, tag="mx")
        nc.tensor.transpose(o_ps[:, :], o_sb[:, 0:1], id_sb[0:CR, 0:CR])
        nc.vector.scalar_tensor_tensor(
            out=o_row, in0=o_ps[0:1, :], scalar=1.0, in1=pb_row[0:1, :],
            op0=OP.mult, op1=OP.add,
        )
        nc.vector.tensor_scalar_max(out=o_row, in0=o_row, scalar1=0.0)
        nc.sync.dma_start(out=out_d[None, :], in_=o_row)

    return nc


def _get_nc() -> bass.Bass:
    if "nc" not in _CACHE:
        _CACHE["nc"] = _build_nc()
    return _CACHE["nc"]


def _ensure_ntff_hook():
    """The image's antenv package lacks axon_hooks; shim it so
    run_bass_kernel_spmd(trace=True) can reach the NTFF profiler."""
    import types

    if "antenv.axon_hooks" in sys.modules:
        return
    m = types.ModuleType("antenv.axon_hooks")
    _hook = [None]
    m.set_axon_ntff_profile_hook = lambda h: _hook.__setitem__(0, h)
    m.get_axon_ntff_profile_hook = lambda: _hook[0]
    sys.modules["antenv.axon_hooks"] = m
    try:
        import antenv

        antenv.axon_hooks = m
        from trn_agent_boot.trn_boot import _ntff_profile_via_ctypes

        m.set_axon_ntff_profile_hook(
            _ntff_profile_via_ctypes("/opt/axon/libaxon_pjrt.so")
        )
    except Exception:
        pass


def _run(x, proj_w, proj_b, trace=False):
    if trace:
        _ensure_ntff_hook()
    nc = _get_nc()
    in_maps = [
        {
            "x": np.ascontiguousarray(x[b], dtype=np.float32),
            "proj_w": np.ascontiguousarray(proj_w, dtype=np.float32),
            "proj_b": np.ascontiguousarray(proj_b, dtype=np.float32),
        }
        for b in range(B)
    ]
    res = run_bass_kernel_spmd(nc, in_maps, list(range(B)), trace=trace)
    out = np.stack([res.results[b]["out"].reshape(1, CR) for b in range(B)])
    return out.astype(np.float32), res


def kernel(x, ln_w, ln_b, proj_w, proj_b):
    x = np.asarray(x)
    ln_w = np.asarray(ln_w)
    ln_b = np.asarray(ln_b)
    proj_w = np.asarray(proj_w)
    proj_b = np.asarray(proj_b)
    if not (np.allclose(ln_w, 1.0) and np.allclose(ln_b, 0.0)):
        # General ln_w/ln_b fallback (never hit with the spec's fills: ones/zeros).
        return _kernel_numpy(x, ln_w, ln_b, proj_w, proj_b)
    out, _ = _run(x, proj_w, proj_b, trace=False)
    return out


def _kernel_numpy(x, ln_w, ln_b, proj_w, proj_b):
    x = x.astype(np.float32)
    mu = x.mean(-1, keepdims=True)
    var = x.var(-1, keepdims=True)
    xn = (x - mu) / np.sqrt(var + LN_EPS) * ln_w + ln_b
    nrm = np.linalg.norm(xn, axis=-1, keepdims=True)
    out = []
    for b in range(x.shape[0]):
        cos = (xn[b] @ xn[b].T) / (nrm[b] @ nrm[b].T + 1e-8)
        den = cos.sum(-1)
        mask = (den == den.max()).astype(np.float32)[:, None]
        center = (xn[b] * mask).sum(0)
        out.append(np.maximum(proj_w @ center + proj_b, 0.0))
    return np.stack(out)[:, None, :].astype(np.float32)
